# revision 9
# baseline (speedup 1.0000x reference)
"""CircleLoss kernel for 8 Trainium2 NeuronCores (Bass/Tile).

Data-parallel over rows: each core processes N/8 = 8192 rows of the
(65536, 512) anchor/positive/negative tensors, computes row-wise dots
with fused DVE tensor_tensor_reduce, the per-row exp terms on ACT, and
per-partition partial sums of similarity_pos / similarity_neg. The host
sums the 8x[128,2] partials and applies the final log.
"""

import numpy as np

import concourse.bass as bass
import concourse.tile as tile
from concourse import mybir
from concourse.bass_utils import run_bass_kernel_spmd

N_TOTAL = 65536
D = 512
N_CORES = 8
ROWS = N_TOTAL // N_CORES  # 8192 rows per core
P = 128                    # SBUF partitions
GAMMA = 1.0

F32 = mybir.dt.float32

# Set by an external test harness to request an NTFF profile; the
# measured on-device time lands in LAST_EXEC_NS.
TRACE = False
LAST_EXEC_NS = None

_PROGRAM_CACHE = {}


def _split_multi_waits(nc):
    """Split instructions with >1 sync wait into single-wait NoOps.

    The walrus codegen in this image only encodes one sync wait per
    instruction ("Too many sync wait commands"); the tile scheduler can
    attach several. Engine streams execute in order, so hoisting all but
    the last wait onto preceding NoOps is semantics-preserving.
    """
    cnt = 0
    for fn in nc.m.functions:
        for blk in fn.blocks:
            new_list = []
            for ins in blk.instructions:
                si = ins.sync_info
                if si is not None and len(si.on_wait) > 1:
                    waits = list(si.on_wait)
                    for w in waits[:-1]:
                        nop = mybir.InstNoOp(name=f"splitw-{cnt}")
                        cnt += 1
                        nop.engine = ins.engine
                        nop.sync_info = mybir.SyncInfo(on_wait=[w],
                                                       on_update=[])
                        new_list.append(nop)
                    ins.sync_info = mybir.SyncInfo(
                        on_wait=[waits[-1]], on_update=list(si.on_update))
                new_list.append(ins)
            blk.instructions = new_list
    return nc


def _build(margin, rows=ROWS, cb=4, split_waits=True):
    """Build the SPMD Bass program for one core's row shard."""
    m = float(margin)
    delta_p = 1.0 - m
    delta_n = m
    o_p = 1.0 + m
    o_n = -m

    blocks = rows // P          # 128-row blocks per core
    chunks = blocks // cb       # DMA chunks (cb blocks each)
    assert blocks % cb == 0

    nc = bass.Bass("TRN2", target_bir_lowering=False, debug=False,
                   num_devices=N_CORES)

    a = nc.dram_tensor("anchor", [rows, D], F32, kind="ExternalInput").ap()
    p = nc.dram_tensor("positive", [rows, D], F32, kind="ExternalInput").ap()
    n = nc.dram_tensor("negative", [rows, D], F32, kind="ExternalInput").ap()
    out = nc.dram_tensor("partials", [P, 2], F32, kind="ExternalOutput").ap()

    # row index = (c*cb + b)*P + part  ->  view [c][part][b][d]
    av = a.rearrange("(c b p) d -> c p b d", b=cb, p=P)
    pv = p.rearrange("(c b p) d -> c p b d", b=cb, p=P)
    nv = n.rearrange("(c b p) d -> c p b d", b=cb, p=P)

    with tile.TileContext(nc) as tc:
        with (
            tc.tile_pool(name="io", bufs=3) as io,
            tc.tile_pool(name="acc", bufs=1) as acc,
            tc.tile_pool(name="fin", bufs=1) as fin,
        ):
            pos_cos = acc.tile([P, blocks], F32, tag="pos_cos")
            neg_cos = acc.tile([P, blocks], F32, tag="neg_cos")
            # tensor_tensor_reduce must write its elementwise product
            # somewhere; it is never read back.
            scratch = acc.tile([P, D], F32, tag="scratch")

            for c in range(chunks):
                a_t = io.tile([P, cb, D], F32, tag="a")
                p_t = io.tile([P, cb, D], F32, tag="p")
                n_t = io.tile([P, cb, D], F32, tag="n")
                nc.sync.dma_start(out=a_t, in_=av[c])
                nc.sync.dma_start(out=p_t, in_=pv[c])
                nc.sync.dma_start(out=n_t, in_=nv[c])
                for b in range(cb):
                    blk = c * cb + b
                    # out = (a bypass _) * p ; accum = row-wise dot
                    nc.vector.scalar_tensor_tensor(
                        out=scratch,
                        in0=a_t[:, b, :],
                        scalar=0.0,
                        in1=p_t[:, b, :],
                        op0=mybir.AluOpType.bypass,
                        op1=mybir.AluOpType.mult,
                        accum_out=pos_cos[:, blk:blk + 1],
                    )
                    nc.vector.scalar_tensor_tensor(
                        out=scratch,
                        in0=a_t[:, b, :],
                        scalar=0.0,
                        in1=n_t[:, b, :],
                        op0=mybir.AluOpType.bypass,
                        op1=mybir.AluOpType.mult,
                        accum_out=neg_cos[:, blk:blk + 1],
                    )

            # Per-row finisher on [P, blocks] tiles.
            bias_op = fin.tile([P, 1], F32, tag="bias_op")
            bias_mn = fin.tile([P, 1], F32, tag="bias_mn")
            nc.vector.memset(bias_op, o_p)
            nc.vector.memset(bias_mn, -o_n)

            alpha_p = fin.tile([P, blocks], F32, tag="alpha_p")
            alpha_n = fin.tile([P, blocks], F32, tag="alpha_n")
            arg_p = fin.tile([P, blocks], F32, tag="arg_p")
            arg_n = fin.tile([P, blocks], F32, tag="arg_n")
            sim_p = fin.tile([P, blocks], F32, tag="sim_p")
            sim_n = fin.tile([P, blocks], F32, tag="sim_n")
            partials = fin.tile([P, 2], F32, tag="partials")

            # alpha_p = |o_p - pos_cos|
            nc.scalar.activation(out=alpha_p, in_=pos_cos,
                                 func=mybir.ActivationFunctionType.Abs,
                                 bias=bias_op, scale=-1.0)
            # arg_p = (pos_cos - delta_p) * alpha_p
            nc.vector.scalar_tensor_tensor(out=arg_p, in0=pos_cos,
                                           scalar=delta_p, in1=alpha_p,
                                           op0=mybir.AluOpType.subtract,
                                           op1=mybir.AluOpType.mult)
            # sim_p = exp(-gamma * arg_p); partials[:,0] = sum(sim_p)
            nc.scalar.activation(out=sim_p, in_=arg_p,
                                 func=mybir.ActivationFunctionType.Exp,
                                 scale=-GAMMA,
                                 accum_out=partials[:, 0:1])

            # alpha_n = |neg_cos - o_n|
            nc.scalar.activation(out=alpha_n, in_=neg_cos,
                                 func=mybir.ActivationFunctionType.Abs,
                                 bias=bias_mn, scale=1.0)
            # arg_n = (neg_cos - delta_n) * alpha_n
            nc.vector.scalar_tensor_tensor(out=arg_n, in0=neg_cos,
                                           scalar=delta_n, in1=alpha_n,
                                           op0=mybir.AluOpType.subtract,
                                           op1=mybir.AluOpType.mult)
            # sim_n = exp(gamma * arg_n); partials[:,1] = sum(sim_n)
            nc.scalar.activation(out=sim_n, in_=arg_n,
                                 func=mybir.ActivationFunctionType.Exp,
                                 scale=GAMMA,
                                 accum_out=partials[:, 1:2])

            nc.sync.dma_start(out=out, in_=partials)

    return _split_multi_waits(nc) if split_waits else nc


def kernel(**inputs):
    global LAST_EXEC_NS
    a = np.ascontiguousarray(np.asarray(inputs["anchor"], dtype=np.float32))
    p = np.ascontiguousarray(np.asarray(inputs["positive"], dtype=np.float32))
    n = np.ascontiguousarray(np.asarray(inputs["negative"], dtype=np.float32))
    margin = float(np.asarray(inputs["margin"]))
    assert a.shape == (N_TOTAL, D)

    key = margin
    if key not in _PROGRAM_CACHE:
        _PROGRAM_CACHE[key] = _build(margin)
    nc = _PROGRAM_CACHE[key]

    in_maps = [
        {
            "anchor": a[i * ROWS:(i + 1) * ROWS],
            "positive": p[i * ROWS:(i + 1) * ROWS],
            "negative": n[i * ROWS:(i + 1) * ROWS],
        }
        for i in range(N_CORES)
    ]
    res = run_bass_kernel_spmd(nc, in_maps, core_ids=list(range(N_CORES)),
                               trace=TRACE)
    LAST_EXEC_NS = res.exec_time_ns
    globals()["LAST_RESULTS"] = res

    parts = np.stack([r["partials"] for r in res.results])  # [8, 128, 2]
    sp = np.float32(parts[:, :, 0].astype(np.float32).sum(dtype=np.float32))
    sn = np.float32(parts[:, :, 1].astype(np.float32).sum(dtype=np.float32))

    # Mirror the jax-on-neuron reference: a reduce over values containing
    # inf yields nan there, so a non-finite partial sum maps to nan.
    if not (np.isfinite(sp) and np.isfinite(sn)):
        return np.float32(np.nan)
    prod = np.float32(sp) * np.float32(sn)
    return np.float32(np.log1p(prod))


# revision 22
# speedup vs baseline: 1.0390x; 1.0390x over previous
"""CircleLoss kernel for 8 Trainium2 NeuronCores (Bass/Tile).

Data-parallel over rows: each core processes N/8 = 8192 rows of the
(65536, 512) anchor/positive/negative tensors, computes row-wise dots
with fused DVE scalar_tensor_tensor (multiply + free-dim accumulate in
one pass), the per-row exp terms on ACT (Exp with accum_out), and
per-partition partial sums of similarity_pos / similarity_neg. The host
sums the 8x[128,2] partials and applies the final log.
"""

import os

import numpy as np

import concourse.bass as bass
import concourse.tile as tile
from concourse import mybir
from concourse.bass_utils import run_bass_kernel_spmd

N_TOTAL = 65536
D = 512
N_CORES = 8
ROWS = N_TOTAL // N_CORES  # 8192 rows per core
P = 128                    # SBUF partitions
GAMMA = 1.0

F32 = mybir.dt.float32

# Set by an external test harness to request an NTFF profile; the
# measured on-device time lands in LAST_EXEC_NS.
TRACE = False
LAST_EXEC_NS = None

_PROGRAM_CACHE = {}


def _split_multi_waits(nc):
    """Split instructions with >1 sync wait into single-wait NoOps.

    The walrus codegen in this image only encodes one sync wait per
    instruction ("Too many sync wait commands"); the tile scheduler can
    attach several. Engine streams execute in order, so hoisting all but
    the last wait onto preceding NoOps is semantics-preserving.
    """
    cnt = 0
    for fn in nc.m.functions:
        for blk in fn.blocks:
            new_list = []
            for ins in blk.instructions:
                si = ins.sync_info
                if si is not None and len(si.on_wait) > 1:
                    waits = list(si.on_wait)
                    for w in waits[:-1]:
                        nop = mybir.InstNoOp(name=f"splitw-{cnt}")
                        cnt += 1
                        nop.engine = ins.engine
                        nop.sync_info = mybir.SyncInfo(on_wait=[w],
                                                       on_update=[])
                        new_list.append(nop)
                    ins.sync_info = mybir.SyncInfo(
                        on_wait=[waits[-1]], on_update=list(si.on_update))
                new_list.append(ins)
            blk.instructions = new_list
    return nc


def _build(margin, rows=ROWS, cb=4, io_bufs=3, multi_ring=False,
           pace=0, split_waits=True):
    """Build the SPMD Bass program for one core's row shard."""
    m = float(margin)
    delta_p = 1.0 - m
    delta_n = m
    o_p = 1.0 + m
    o_n = -m

    blocks = rows // P          # rows handled per partition
    chunks = blocks // cb       # DMA chunks (cb rows per partition each)
    assert blocks % cb == 0

    nc = bass.Bass("TRN2", target_bir_lowering=False, debug=False,
                   num_devices=N_CORES)

    a = nc.dram_tensor("anchor", [rows, D], F32, kind="ExternalInput").ap()
    p = nc.dram_tensor("positive", [rows, D], F32, kind="ExternalInput").ap()
    n = nc.dram_tensor("negative", [rows, D], F32, kind="ExternalInput").ap()
    out = nc.dram_tensor("partials", [P, 2], F32, kind="ExternalOutput").ap()

    # row index = part*blocks + r: each partition streams a contiguous
    # row range, so every DMA moves cb*D*4 contiguous bytes per partition.
    av = a.rearrange("(p r) d -> p r d", p=P)
    pv = p.rearrange("(p r) d -> p r d", p=P)
    nv = n.rearrange("(p r) d -> p r d", p=P)

    with tile.TileContext(nc) as tc:
        with (
            tc.tile_pool(name="io", bufs=io_bufs) as io,
            tc.tile_pool(name="acc", bufs=1) as acc,
            tc.tile_pool(name="fin", bufs=1) as fin,
        ):
            pos_cos = acc.tile([P, blocks], F32, tag="pos_cos")
            neg_cos = acc.tile([P, blocks], F32, tag="neg_cos")
            junk = acc.tile([P, 1], F32, tag="junk")
            # tensor_tensor_reduce must write its elementwise product
            # somewhere; it is never read back.
            scratch = acc.tile([P, D], F32, tag="scratch")

            for c in range(chunks):
                a_t = io.tile([P, cb, D], F32, tag="a")
                p_t = io.tile([P, cb, D], F32, tag="p")
                n_t = io.tile([P, cb, D], F32, tag="n")
                eng_a = nc.sync
                eng_p = nc.scalar if multi_ring else nc.sync
                eng_n = nc.gpsimd if multi_ring else nc.sync
                eng_a.dma_start(out=a_t, in_=av[:, c * cb:(c + 1) * cb, :])
                eng_p.dma_start(out=p_t, in_=pv[:, c * cb:(c + 1) * cb, :])
                eng_n.dma_start(out=n_t, in_=nv[:, c * cb:(c + 1) * cb, :])
                for b in range(cb):
                    blk = c * cb + b
                    # out = (a bypass _) * p ; accum = row-wise dot
                    nc.vector.scalar_tensor_tensor(
                        out=scratch,
                        in0=a_t[:, b, :],
                        scalar=0.0,
                        in1=p_t[:, b, :],
                        op0=mybir.AluOpType.bypass,
                        op1=mybir.AluOpType.mult,
                        accum_out=pos_cos[:, blk:blk + 1],
                    )
                    nc.vector.scalar_tensor_tensor(
                        out=scratch,
                        in0=a_t[:, b, :],
                        scalar=0.0,
                        in1=n_t[:, b, :],
                        op0=mybir.AluOpType.bypass,
                        op1=mybir.AluOpType.mult,
                        accum_out=neg_cos[:, blk:blk + 1],
                    )
                # Optional DVE pacing: keeps this core's HBM demand near
                # the per-stack fair share so paired cores split bandwidth
                # evenly instead of racing (lowers the max-over-cores time).
                for _ in range(pace):
                    nc.vector.scalar_tensor_tensor(
                        out=scratch,
                        in0=a_t[:, 0, :],
                        scalar=0.0,
                        in1=p_t[:, 0, :],
                        op0=mybir.AluOpType.bypass,
                        op1=mybir.AluOpType.mult,
                        accum_out=junk,
                    )

            # Per-row finisher on [P, blocks] tiles.
            bias_op = fin.tile([P, 1], F32, tag="bias_op")
            bias_mn = fin.tile([P, 1], F32, tag="bias_mn")
            nc.vector.memset(bias_op, o_p)
            nc.vector.memset(bias_mn, -o_n)

            alpha_p = fin.tile([P, blocks], F32, tag="alpha_p")
            alpha_n = fin.tile([P, blocks], F32, tag="alpha_n")
            arg_p = fin.tile([P, blocks], F32, tag="arg_p")
            arg_n = fin.tile([P, blocks], F32, tag="arg_n")
            sim_p = fin.tile([P, blocks], F32, tag="sim_p")
            sim_n = fin.tile([P, blocks], F32, tag="sim_n")
            partials = fin.tile([P, 2], F32, tag="partials")

            # alpha_p = |o_p - pos_cos|
            nc.scalar.activation(out=alpha_p, in_=pos_cos,
                                 func=mybir.ActivationFunctionType.Abs,
                                 bias=bias_op, scale=-1.0)
            # arg_p = (pos_cos - delta_p) * alpha_p
            nc.vector.scalar_tensor_tensor(out=arg_p, in0=pos_cos,
                                           scalar=delta_p, in1=alpha_p,
                                           op0=mybir.AluOpType.subtract,
                                           op1=mybir.AluOpType.mult)
            # sim_p = exp(-gamma * arg_p); partials[:,0] = sum(sim_p)
            nc.scalar.activation(out=sim_p, in_=arg_p,
                                 func=mybir.ActivationFunctionType.Exp,
                                 scale=-GAMMA,
                                 accum_out=partials[:, 0:1])

            # alpha_n = |neg_cos - o_n|
            nc.scalar.activation(out=alpha_n, in_=neg_cos,
                                 func=mybir.ActivationFunctionType.Abs,
                                 bias=bias_mn, scale=1.0)
            # arg_n = (neg_cos - delta_n) * alpha_n
            nc.vector.scalar_tensor_tensor(out=arg_n, in0=neg_cos,
                                           scalar=delta_n, in1=alpha_n,
                                           op0=mybir.AluOpType.subtract,
                                           op1=mybir.AluOpType.mult)
            # sim_n = exp(gamma * arg_n); partials[:,1] = sum(sim_n)
            nc.scalar.activation(out=sim_n, in_=arg_n,
                                 func=mybir.ActivationFunctionType.Exp,
                                 scale=GAMMA,
                                 accum_out=partials[:, 1:2])

            nc.sync.dma_start(out=out, in_=partials)

    return _split_multi_waits(nc) if split_waits else nc


def kernel(**inputs):
    global LAST_EXEC_NS
    a = np.ascontiguousarray(np.asarray(inputs["anchor"], dtype=np.float32))
    p = np.ascontiguousarray(np.asarray(inputs["positive"], dtype=np.float32))
    n = np.ascontiguousarray(np.asarray(inputs["negative"], dtype=np.float32))
    margin = float(np.asarray(inputs["margin"]))
    assert a.shape == (N_TOTAL, D)

    key = margin
    if key not in _PROGRAM_CACHE:
        _PROGRAM_CACHE[key] = _build(margin, cb=2, io_bufs=6)
    nc = _PROGRAM_CACHE[key]

    in_maps = [
        {
            "anchor": a[i * ROWS:(i + 1) * ROWS],
            "positive": p[i * ROWS:(i + 1) * ROWS],
            "negative": n[i * ROWS:(i + 1) * ROWS],
        }
        for i in range(N_CORES)
    ]
    # NTFF tracing needs an axon profile hook that not every container
    # registers; hard-disable env-triggered tracing unless requested.
    if TRACE:
        os.environ.pop("BASS_NEVER_TRACE", None)
    else:
        os.environ["BASS_NEVER_TRACE"] = "1"
    res = run_bass_kernel_spmd(nc, in_maps, core_ids=list(range(N_CORES)),
                               trace=TRACE)
    LAST_EXEC_NS = res.exec_time_ns
    globals()["LAST_RESULTS"] = res

    parts = np.stack([r["partials"] for r in res.results])  # [8, 128, 2]
    sp = np.float32(parts[:, :, 0].astype(np.float32).sum(dtype=np.float32))
    sn = np.float32(parts[:, :, 1].astype(np.float32).sum(dtype=np.float32))

    # Mirror the jax-on-neuron reference: a reduce over values containing
    # inf yields nan there, so a non-finite partial sum maps to nan.
    if not (np.isfinite(sp) and np.isfinite(sn)):
        return np.float32(np.nan)
    prod = np.float32(sp) * np.float32(sn)
    return np.float32(np.log1p(prod))


# revision 23
# speedup vs baseline: 1.1061x; 1.0646x over previous
"""CircleLoss kernel for 8 Trainium2 NeuronCores (Bass/Tile).

Data-parallel over rows: each core processes N/8 = 8192 rows of the
(65536, 512) anchor/positive/negative tensors, computes row-wise dots
with fused DVE scalar_tensor_tensor (multiply + free-dim accumulate in
one pass), the per-row exp terms on ACT (Exp with accum_out), and
per-partition partial sums of similarity_pos / similarity_neg. The host
sums the 8x[128,2] partials and applies the final log.
"""

import os

import numpy as np

import concourse.bass as bass
import concourse.tile as tile
from concourse import mybir
from concourse.bass_utils import run_bass_kernel_spmd

N_TOTAL = 65536
D = 512
N_CORES = 8
ROWS = N_TOTAL // N_CORES  # 8192 rows per core
P = 128                    # SBUF partitions
GAMMA = 1.0

F32 = mybir.dt.float32

# Set by an external test harness to request an NTFF profile; the
# measured on-device time lands in LAST_EXEC_NS.
TRACE = False
LAST_EXEC_NS = None

_PROGRAM_CACHE = {}


def _split_multi_waits(nc):
    """Split instructions with >1 sync wait into single-wait NoOps.

    The walrus codegen in this image only encodes one sync wait per
    instruction ("Too many sync wait commands"); the tile scheduler can
    attach several. Engine streams execute in order, so hoisting all but
    the last wait onto preceding NoOps is semantics-preserving.
    """
    cnt = 0
    for fn in nc.m.functions:
        for blk in fn.blocks:
            new_list = []
            for ins in blk.instructions:
                si = ins.sync_info
                if si is not None and len(si.on_wait) > 1:
                    waits = list(si.on_wait)
                    for w in waits[:-1]:
                        nop = mybir.InstNoOp(name=f"splitw-{cnt}")
                        cnt += 1
                        nop.engine = ins.engine
                        nop.sync_info = mybir.SyncInfo(on_wait=[w],
                                                       on_update=[])
                        new_list.append(nop)
                    ins.sync_info = mybir.SyncInfo(
                        on_wait=[waits[-1]], on_update=list(si.on_update))
                new_list.append(ins)
            blk.instructions = new_list
    return nc


def _build(margin, rows=ROWS, cb=4, io_bufs=3, multi_ring=False,
           pace=0, split_waits=True):
    """Build the SPMD Bass program for one core's row shard."""
    m = float(margin)
    delta_p = 1.0 - m
    delta_n = m
    o_p = 1.0 + m
    o_n = -m

    blocks = rows // P          # rows handled per partition
    chunks = blocks // cb       # DMA chunks (cb rows per partition each)
    assert blocks % cb == 0

    nc = bass.Bass("TRN2", target_bir_lowering=False, debug=False,
                   num_devices=N_CORES)

    a = nc.dram_tensor("anchor", [rows, D], F32, kind="ExternalInput").ap()
    p = nc.dram_tensor("positive", [rows, D], F32, kind="ExternalInput").ap()
    n = nc.dram_tensor("negative", [rows, D], F32, kind="ExternalInput").ap()
    out = nc.dram_tensor("partials", [P, 2], F32, kind="ExternalOutput").ap()

    # row index = part*blocks + r: each partition streams a contiguous
    # row range, so every DMA moves cb*D*4 contiguous bytes per partition.
    av = a.rearrange("(p r) d -> p r d", p=P)
    pv = p.rearrange("(p r) d -> p r d", p=P)
    nv = n.rearrange("(p r) d -> p r d", p=P)

    with tile.TileContext(nc) as tc:
        with (
            tc.tile_pool(name="io", bufs=io_bufs) as io,
            tc.tile_pool(name="acc", bufs=1) as acc,
            tc.tile_pool(name="fin", bufs=1) as fin,
        ):
            pos_cos = acc.tile([P, blocks], F32, tag="pos_cos")
            neg_cos = acc.tile([P, blocks], F32, tag="neg_cos")
            junk = acc.tile([P, 1], F32, tag="junk")
            # tensor_tensor_reduce must write its elementwise product
            # somewhere; it is never read back.
            scratch = acc.tile([P, D], F32, tag="scratch")

            for c in range(chunks):
                a_t = io.tile([P, cb, D], F32, tag="a")
                p_t = io.tile([P, cb, D], F32, tag="p")
                n_t = io.tile([P, cb, D], F32, tag="n")
                eng_a = nc.sync
                eng_p = nc.scalar if multi_ring else nc.sync
                eng_n = nc.gpsimd if multi_ring else nc.sync
                eng_a.dma_start(out=a_t, in_=av[:, c * cb:(c + 1) * cb, :])
                eng_p.dma_start(out=p_t, in_=pv[:, c * cb:(c + 1) * cb, :])
                eng_n.dma_start(out=n_t, in_=nv[:, c * cb:(c + 1) * cb, :])
                for b in range(cb):
                    blk = c * cb + b
                    # out = (a bypass _) * p ; accum = row-wise dot
                    nc.vector.scalar_tensor_tensor(
                        out=scratch,
                        in0=a_t[:, b, :],
                        scalar=0.0,
                        in1=p_t[:, b, :],
                        op0=mybir.AluOpType.bypass,
                        op1=mybir.AluOpType.mult,
                        accum_out=pos_cos[:, blk:blk + 1],
                    )
                    nc.vector.scalar_tensor_tensor(
                        out=scratch,
                        in0=a_t[:, b, :],
                        scalar=0.0,
                        in1=n_t[:, b, :],
                        op0=mybir.AluOpType.bypass,
                        op1=mybir.AluOpType.mult,
                        accum_out=neg_cos[:, blk:blk + 1],
                    )
                # Optional DVE pacing: keeps this core's HBM demand near
                # the per-stack fair share so paired cores split bandwidth
                # evenly instead of racing (lowers the max-over-cores time).
                for _ in range(pace):
                    nc.vector.scalar_tensor_tensor(
                        out=scratch,
                        in0=a_t[:, 0, :],
                        scalar=0.0,
                        in1=p_t[:, 0, :],
                        op0=mybir.AluOpType.bypass,
                        op1=mybir.AluOpType.mult,
                        accum_out=junk,
                    )

            # Per-row finisher on [P, blocks] tiles.
            bias_op = fin.tile([P, 1], F32, tag="bias_op")
            bias_mn = fin.tile([P, 1], F32, tag="bias_mn")
            nc.vector.memset(bias_op, o_p)
            nc.vector.memset(bias_mn, -o_n)

            alpha_p = fin.tile([P, blocks], F32, tag="alpha_p")
            alpha_n = fin.tile([P, blocks], F32, tag="alpha_n")
            arg_p = fin.tile([P, blocks], F32, tag="arg_p")
            arg_n = fin.tile([P, blocks], F32, tag="arg_n")
            sim_p = fin.tile([P, blocks], F32, tag="sim_p")
            sim_n = fin.tile([P, blocks], F32, tag="sim_n")
            partials = fin.tile([P, 2], F32, tag="partials")

            # alpha_p = |o_p - pos_cos|
            nc.scalar.activation(out=alpha_p, in_=pos_cos,
                                 func=mybir.ActivationFunctionType.Abs,
                                 bias=bias_op, scale=-1.0)
            # arg_p = (pos_cos - delta_p) * alpha_p
            nc.vector.scalar_tensor_tensor(out=arg_p, in0=pos_cos,
                                           scalar=delta_p, in1=alpha_p,
                                           op0=mybir.AluOpType.subtract,
                                           op1=mybir.AluOpType.mult)
            # sim_p = exp(-gamma * arg_p); partials[:,0] = sum(sim_p)
            nc.scalar.activation(out=sim_p, in_=arg_p,
                                 func=mybir.ActivationFunctionType.Exp,
                                 scale=-GAMMA,
                                 accum_out=partials[:, 0:1])

            # alpha_n = |neg_cos - o_n|
            nc.scalar.activation(out=alpha_n, in_=neg_cos,
                                 func=mybir.ActivationFunctionType.Abs,
                                 bias=bias_mn, scale=1.0)
            # arg_n = (neg_cos - delta_n) * alpha_n
            nc.vector.scalar_tensor_tensor(out=arg_n, in0=neg_cos,
                                           scalar=delta_n, in1=alpha_n,
                                           op0=mybir.AluOpType.subtract,
                                           op1=mybir.AluOpType.mult)
            # sim_n = exp(gamma * arg_n); partials[:,1] = sum(sim_n)
            nc.scalar.activation(out=sim_n, in_=arg_n,
                                 func=mybir.ActivationFunctionType.Exp,
                                 scale=GAMMA,
                                 accum_out=partials[:, 1:2])

            nc.sync.dma_start(out=out, in_=partials)

    return _split_multi_waits(nc) if split_waits else nc


def kernel(**inputs):
    global LAST_EXEC_NS
    a = np.ascontiguousarray(np.asarray(inputs["anchor"], dtype=np.float32))
    p = np.ascontiguousarray(np.asarray(inputs["positive"], dtype=np.float32))
    n = np.ascontiguousarray(np.asarray(inputs["negative"], dtype=np.float32))
    margin = float(np.asarray(inputs["margin"]))
    assert a.shape == (N_TOTAL, D)

    key = margin
    if key not in _PROGRAM_CACHE:
        _PROGRAM_CACHE[key] = _build(margin, cb=4, io_bufs=3)
    nc = _PROGRAM_CACHE[key]

    in_maps = [
        {
            "anchor": a[i * ROWS:(i + 1) * ROWS],
            "positive": p[i * ROWS:(i + 1) * ROWS],
            "negative": n[i * ROWS:(i + 1) * ROWS],
        }
        for i in range(N_CORES)
    ]
    # NTFF tracing needs an axon profile hook that not every container
    # registers; hard-disable env-triggered tracing unless requested.
    if TRACE:
        os.environ.pop("BASS_NEVER_TRACE", None)
    else:
        os.environ["BASS_NEVER_TRACE"] = "1"
    res = run_bass_kernel_spmd(nc, in_maps, core_ids=list(range(N_CORES)),
                               trace=TRACE)
    LAST_EXEC_NS = res.exec_time_ns
    globals()["LAST_RESULTS"] = res

    parts = np.stack([r["partials"] for r in res.results])  # [8, 128, 2]
    sp = np.float32(parts[:, :, 0].astype(np.float32).sum(dtype=np.float32))
    sn = np.float32(parts[:, :, 1].astype(np.float32).sum(dtype=np.float32))

    # Mirror the jax-on-neuron reference: a reduce over values containing
    # inf yields nan there, so a non-finite partial sum maps to nan.
    if not (np.isfinite(sp) and np.isfinite(sn)):
        return np.float32(np.nan)
    prod = np.float32(sp) * np.float32(sn)
    return np.float32(np.log1p(prod))


# revision 28
# speedup vs baseline: 1.1333x; 1.0246x over previous
"""CircleLoss kernel for 8 Trainium2 NeuronCores (Bass/Tile).

Data-parallel over rows: each core processes N/8 = 8192 rows of the
(65536, 512) anchor/positive/negative tensors, computes row-wise dots
with fused DVE scalar_tensor_tensor (multiply + free-dim accumulate in
one pass), the per-row exp terms on ACT (Exp with accum_out), and
per-partition partial sums of similarity_pos / similarity_neg. The host
sums the 8x[128,2] partials and applies the final log.
"""

import os

import numpy as np

import concourse.bass as bass
import concourse.tile as tile
from concourse import mybir
from concourse.bass_utils import run_bass_kernel_spmd

N_TOTAL = 65536
D = 512
N_CORES = 8
ROWS = N_TOTAL // N_CORES  # 8192 rows per core
P = 128                    # SBUF partitions
GAMMA = 1.0

F32 = mybir.dt.float32

# Set by an external test harness to request an NTFF profile; the
# measured on-device time lands in LAST_EXEC_NS.
TRACE = False
LAST_EXEC_NS = None

_PROGRAM_CACHE = {}


def _split_multi_waits(nc):
    """Split instructions with >1 sync wait into single-wait NoOps.

    The walrus codegen in this image only encodes one sync wait per
    instruction ("Too many sync wait commands"); the tile scheduler can
    attach several. Engine streams execute in order, so hoisting all but
    the last wait onto preceding NoOps is semantics-preserving.
    """
    cnt = 0
    for fn in nc.m.functions:
        for blk in fn.blocks:
            new_list = []
            for ins in blk.instructions:
                si = ins.sync_info
                if si is not None and len(si.on_wait) > 1:
                    waits = list(si.on_wait)
                    for w in waits[:-1]:
                        nop = mybir.InstNoOp(name=f"splitw-{cnt}")
                        cnt += 1
                        nop.engine = ins.engine
                        nop.sync_info = mybir.SyncInfo(on_wait=[w],
                                                       on_update=[])
                        new_list.append(nop)
                    ins.sync_info = mybir.SyncInfo(
                        on_wait=[waits[-1]], on_update=list(si.on_update))
                new_list.append(ins)
            blk.instructions = new_list
    return nc


def _build(margin, rows=ROWS, cb=4, io_bufs=3, multi_ring=False,
           pace=0, taper=False, split_waits=True):
    """Build the SPMD Bass program for one core's row shard."""
    m = float(margin)
    delta_p = 1.0 - m
    delta_n = m
    o_p = 1.0 + m
    o_n = -m

    blocks = rows // P          # rows handled per partition
    assert blocks % cb == 0
    if taper and blocks >= 4 * cb and cb > 1:
        # Small chunks at both ends: the first chunk's compute can start
        # after a fraction of the DMA bytes, and the last chunk leaves
        # only a small compute bite after the stream drains.
        sizes = [1, 1, 2] + [cb] * ((blocks - 8) // cb) + [2, 1, 1]
        assert sum(sizes) == blocks
    else:
        sizes = [cb] * (blocks // cb)

    nc = bass.Bass("TRN2", target_bir_lowering=False, debug=False,
                   num_devices=N_CORES)

    a = nc.dram_tensor("anchor", [rows, D], F32, kind="ExternalInput").ap()
    p = nc.dram_tensor("positive", [rows, D], F32, kind="ExternalInput").ap()
    n = nc.dram_tensor("negative", [rows, D], F32, kind="ExternalInput").ap()
    out = nc.dram_tensor("partials", [P, 2], F32, kind="ExternalOutput").ap()

    # row index = part*blocks + r: each partition streams a contiguous
    # row range, so every DMA moves cb*D*4 contiguous bytes per partition.
    av = a.rearrange("(p r) d -> p r d", p=P)
    pv = p.rearrange("(p r) d -> p r d", p=P)
    nv = n.rearrange("(p r) d -> p r d", p=P)

    with tile.TileContext(nc) as tc:
        with (
            tc.tile_pool(name="io", bufs=io_bufs) as io,
            tc.tile_pool(name="acc", bufs=1) as acc,
            tc.tile_pool(name="fin", bufs=1) as fin,
        ):
            pos_cos = acc.tile([P, blocks], F32, tag="pos_cos")
            neg_cos = acc.tile([P, blocks], F32, tag="neg_cos")
            junk = acc.tile([P, 1], F32, tag="junk")
            # tensor_tensor_reduce must write its elementwise product
            # somewhere; it is never read back.
            scratch = acc.tile([P, D], F32, tag="scratch")

            off = 0
            for csz in sizes:
                a_t = io.tile([P, csz, D], F32, tag="a")
                p_t = io.tile([P, csz, D], F32, tag="p")
                n_t = io.tile([P, csz, D], F32, tag="n")
                eng_a = nc.sync
                eng_p = nc.scalar if multi_ring else nc.sync
                eng_n = nc.gpsimd if multi_ring else nc.sync
                eng_a.dma_start(out=a_t, in_=av[:, off:off + csz, :])
                eng_p.dma_start(out=p_t, in_=pv[:, off:off + csz, :])
                eng_n.dma_start(out=n_t, in_=nv[:, off:off + csz, :])
                for b in range(csz):
                    blk = off + b
                    # out = (a bypass _) * p ; accum = row-wise dot
                    nc.vector.scalar_tensor_tensor(
                        out=scratch,
                        in0=a_t[:, b, :],
                        scalar=0.0,
                        in1=p_t[:, b, :],
                        op0=mybir.AluOpType.bypass,
                        op1=mybir.AluOpType.mult,
                        accum_out=pos_cos[:, blk:blk + 1],
                    )
                    nc.vector.scalar_tensor_tensor(
                        out=scratch,
                        in0=a_t[:, b, :],
                        scalar=0.0,
                        in1=n_t[:, b, :],
                        op0=mybir.AluOpType.bypass,
                        op1=mybir.AluOpType.mult,
                        accum_out=neg_cos[:, blk:blk + 1],
                    )
                off += csz
                # Optional DVE pacing: keeps this core's HBM demand near
                # the per-stack fair share so paired cores split bandwidth
                # evenly instead of racing (lowers the max-over-cores time).
                for _ in range(pace):
                    nc.vector.scalar_tensor_tensor(
                        out=scratch,
                        in0=a_t[:, 0, :],
                        scalar=0.0,
                        in1=p_t[:, 0, :],
                        op0=mybir.AluOpType.bypass,
                        op1=mybir.AluOpType.mult,
                        accum_out=junk,
                    )

            # Per-row finisher on [P, blocks] tiles.
            bias_op = fin.tile([P, 1], F32, tag="bias_op")
            bias_mn = fin.tile([P, 1], F32, tag="bias_mn")
            nc.vector.memset(bias_op, o_p)
            nc.vector.memset(bias_mn, -o_n)

            alpha_p = fin.tile([P, blocks], F32, tag="alpha_p")
            alpha_n = fin.tile([P, blocks], F32, tag="alpha_n")
            arg_p = fin.tile([P, blocks], F32, tag="arg_p")
            arg_n = fin.tile([P, blocks], F32, tag="arg_n")
            sim_p = fin.tile([P, blocks], F32, tag="sim_p")
            sim_n = fin.tile([P, blocks], F32, tag="sim_n")
            partials = fin.tile([P, 2], F32, tag="partials")

            # alpha_p = |o_p - pos_cos|
            nc.scalar.activation(out=alpha_p, in_=pos_cos,
                                 func=mybir.ActivationFunctionType.Abs,
                                 bias=bias_op, scale=-1.0)
            # arg_p = (pos_cos - delta_p) * alpha_p
            nc.vector.scalar_tensor_tensor(out=arg_p, in0=pos_cos,
                                           scalar=delta_p, in1=alpha_p,
                                           op0=mybir.AluOpType.subtract,
                                           op1=mybir.AluOpType.mult)
            # sim_p = exp(-gamma * arg_p); partials[:,0] = sum(sim_p)
            nc.scalar.activation(out=sim_p, in_=arg_p,
                                 func=mybir.ActivationFunctionType.Exp,
                                 scale=-GAMMA,
                                 accum_out=partials[:, 0:1])

            # alpha_n = |neg_cos - o_n|
            nc.scalar.activation(out=alpha_n, in_=neg_cos,
                                 func=mybir.ActivationFunctionType.Abs,
                                 bias=bias_mn, scale=1.0)
            # arg_n = (neg_cos - delta_n) * alpha_n
            nc.vector.scalar_tensor_tensor(out=arg_n, in0=neg_cos,
                                           scalar=delta_n, in1=alpha_n,
                                           op0=mybir.AluOpType.subtract,
                                           op1=mybir.AluOpType.mult)
            # sim_n = exp(gamma * arg_n); partials[:,1] = sum(sim_n)
            nc.scalar.activation(out=sim_n, in_=arg_n,
                                 func=mybir.ActivationFunctionType.Exp,
                                 scale=GAMMA,
                                 accum_out=partials[:, 1:2])

            nc.sync.dma_start(out=out, in_=partials)

    return _split_multi_waits(nc) if split_waits else nc


def kernel(**inputs):
    global LAST_EXEC_NS
    a = np.ascontiguousarray(np.asarray(inputs["anchor"], dtype=np.float32))
    p = np.ascontiguousarray(np.asarray(inputs["positive"], dtype=np.float32))
    n = np.ascontiguousarray(np.asarray(inputs["negative"], dtype=np.float32))
    margin = float(np.asarray(inputs["margin"]))
    assert a.shape == (N_TOTAL, D)

    key = margin
    if key not in _PROGRAM_CACHE:
        _PROGRAM_CACHE[key] = _build(margin, cb=4, io_bufs=3, taper=True)
    nc = _PROGRAM_CACHE[key]

    in_maps = [
        {
            "anchor": a[i * ROWS:(i + 1) * ROWS],
            "positive": p[i * ROWS:(i + 1) * ROWS],
            "negative": n[i * ROWS:(i + 1) * ROWS],
        }
        for i in range(N_CORES)
    ]
    # NTFF tracing needs an axon profile hook that not every container
    # registers; hard-disable env-triggered tracing unless requested.
    if TRACE:
        os.environ.pop("BASS_NEVER_TRACE", None)
    else:
        os.environ["BASS_NEVER_TRACE"] = "1"
    res = run_bass_kernel_spmd(nc, in_maps, core_ids=list(range(N_CORES)),
                               trace=TRACE)
    LAST_EXEC_NS = res.exec_time_ns
    globals()["LAST_RESULTS"] = res

    parts = np.stack([r["partials"] for r in res.results])  # [8, 128, 2]
    sp = np.float32(parts[:, :, 0].astype(np.float32).sum(dtype=np.float32))
    sn = np.float32(parts[:, :, 1].astype(np.float32).sum(dtype=np.float32))

    # Mirror the jax-on-neuron reference: a reduce over values containing
    # inf yields nan there, so a non-finite partial sum maps to nan.
    if not (np.isfinite(sp) and np.isfinite(sn)):
        return np.float32(np.nan)
    prod = np.float32(sp) * np.float32(sn)
    return np.float32(np.log1p(prod))
